# revision 5
# baseline (speedup 1.0000x reference)
"""Trainium2 Bass kernel for nn_CLIP_3v3d_brats (dense_cnn head + gated 1x1 conv).

Sharding: 8 cores = batch(2) x 4 D-slabs of `pred`. The dominant einsum
logits[b,k,:] = sum_c effw[b,k,c]*pred[b,c,:] runs as a block-diagonal
bf16 matmul (4 position groups x 32 channels -> K=128 contraction).

v3 architecture (vs the 128us AllReduce baseline):
- The 2KB x_feat cross-core reduction uses remote_dma_broadcast
  (SBUF->SBUF, XOR-relative dests, sem-signaled) instead of a cold
  ncfw AllReduce: each core broadcasts its [128,4] partial into peer
  slot buffers and tree-sums the 8 slots on arrival (wait rsem>=14).
  Safety: a 1-byte prelude AllGather (bir_kernel_barrier) runs from
  t~0 concurrently with the head compute; the send trigger waits on
  it, so no send can race a slow core's semaphore-clear.
- Head chain has exactly ONE ACT table load: Sqrt is replaced by a
  float-trick Newton rsqrt on DVE, so Square/Relu/Identity/Sigmoid all
  live in one resident set.
- GN stats split scalar/DVE (Square-accum on scalar, add-reduce on
  DVE); window sums use the identity relu(s*x+b) = s*(max(x,-b/s))+b:
  one tensor_scalar(max,accum) pass per slot on DVE for 6 of 8 slots,
  scalar ACT(Relu,scale,bias,accum) for the other 2.
- Stream: 9 column-shifted block-diag stationaries, but tiles run in
  GROUPS (6,4,1,1) with the m-loop outermost inside a group so each
  LDWEIGHTS is reused across the group's PSUM tiles (18 LDW total,
  the rest issue at the pure ~107ns/512col bf16 rate). Dummy matmuls
  pinned after the x_feat partials keep/flip the HAM clock gate ON
  during the exchange wait so the stream runs at 2.4GHz.
- Output copies + DMA are bf16 (halves PSUM-copy write and out wire).
- Small weights packed into 4 tensors on the scalar DMA queue; pred
  tiles stream on sync immediately after xe_slab+xg halves.
"""
import sys
import types

sys.path.insert(0, "/opt/trn_rl_repo")

import numpy as np

try:
    import antenv.axon_hooks  # noqa: F401
except ImportError:
    try:
        import trn_agent_boot.trn_boot as _tb

        _hooks = types.ModuleType("antenv.axon_hooks")
        _the_hook = _tb._ntff_profile_via_ctypes("/opt/axon/libaxon_pjrt.so")
        _hooks.get_axon_ntff_profile_hook = lambda: _the_hook
        _hooks.set_axon_ntff_profile_hook = lambda h: None
        sys.modules["antenv.axon_hooks"] = _hooks
    except Exception:
        pass

from concourse import bacc, tile, mybir
from concourse.bass_utils import run_bass_kernel_spmd

f32 = mybir.dt.float32
bf16 = mybir.dt.bfloat16
u32 = mybir.dt.uint32
AF = mybir.ActivationFunctionType
ALU = mybir.AluOpType

NP_BF16 = mybir.dt.np(bf16)

N_CORES = 8
B = 2
K = 3
EPS = 1e-5
G = 4                      # position groups interleaved on partitions
NPOS = 221184              # positions per core slab: 24*96*96
NG = NPOS // G             # 55296
COLS = 4608                # stream iteration columns (9 matmuls of 512)
NITER = NG // COLS         # 12
NMM = COLS // 512          # 9
MWIDE = 12 * NMM           # 108 output partitions
NSLOT = 4                  # window-sum slots per core
NWIN = 1331                # 11^3 window positions per offset
NSLAB = 1728               # 3*24*24 stats-slab positions per batch
NGRP_ELEMS = 8 * 13824     # elements per (batch, group) for GN stats
NWARM = 12                 # HAM warm-up matmuls during the exchange wait
GROUPS = [(0, 6), (6, 10), (10, 11), (11, 12)]  # stream tile groups

# packed-constant column offsets (f32 [128, *])
PKF_COLS = dict(gstat=0, gnw=2, gapbT=4, msel=8, gsel=10, wseg=14,
                ba2_4=20, bseg=21)
PKF_N = 22
# packed bf16 [128, *]
PKB_COLS = dict(w2dt=0, w_cfT=1024, w_cT=2048, w_a1T=3072, ones6=3104)
PKB_N = 3110
# packed f32 [16, *]: gexp rows0-1 cols0-127, ba1 col 128
PK16F_N = 129
# packed bf16 [16, *]: bcf6 512, id6 6, w_a2T4 128
PK16B_COLS = dict(bcf6=0, id6=512, w_a2T4=518)
PK16B_N = 646

TRACE = False
LAST_EXEC_NS = None
_CACHE = {}


def _build_program():
    nc = bacc.Bacc("TRN2", target_bir_lowering=False, debug=False,
                   num_devices=N_CORES)

    def din(name, shape, dt=f32):
        return nc.dram_tensor(name, shape, dt, kind="ExternalInput").ap()

    pred_s = din("pred_s", [NITER, 128, COLS], bf16)
    xe_slab_d = din("xe_slab", [128, B * NSLAB], bf16)
    xg_d = din("xg", [128, B * NSLOT * NWIN], bf16)
    pkf_d = din("pkf", [128, PKF_N])
    pkb_d = din("pkb", [128, PKB_N], bf16)
    pk16f_d = din("pk16f", [16, PK16F_N])
    pk16b_d = din("pk16b", [16, PK16B_N], bf16)

    out_d = nc.dram_tensor("out_s", [NITER, MWIDE, 512], bf16,
                           kind="ExternalOutput").ap()

    rsem = nc.alloc_semaphore("xrecv_sem")
    lsem = nc.alloc_semaphore("xsend_sem")

    with tile.TileContext(nc) as tc:
        with tc.tile_pool(name="small", bufs=1) as sp, \
             tc.tile_pool(name="hps", bufs=2, space="PSUM") as hps, \
             tc.tile_pool(name="sps", bufs=6, space="PSUM") as sps:
          if True:
            # clear the exchange sems first on gpsimd: safe because no
            # peer can send until its own prelude-AG wait completes,
            # which needs OUR AG arrival, which is after this clear.
            nc.gpsimd.sem_clear(rsem)
            nc.gpsimd.sem_clear(lsem)

            # ---- loads: sync queue = xe_slab, xg halves, pred tiles ----
            xe_slab = sp.tile([128, B * NSLAB], bf16)
            nc.sync.dma_start(xe_slab[:], xe_slab_d[:])
            HW = NSLOT * NWIN
            xg = sp.tile([128, B * HW], bf16)
            nc.sync.dma_start(xg[:, 0:HW], xg_d[:, 0:HW])
            nc.sync.dma_start(xg[:, HW:2 * HW], xg_d[:, HW:2 * HW])
            pts = []
            for t in range(NITER):
                pt = sp.tile([128, COLS], bf16, tag="pt", bufs=NITER)
                nc.sync.dma_start(pt[:], pred_s[t])
                pts.append(pt)
            # scalar queue: packed weights
            pkf = sp.tile([128, PKF_N], f32)
            nc.scalar.dma_start(pkf[:], pkf_d[:])
            pk16f = sp.tile([16, PK16F_N], f32)
            nc.scalar.dma_start(pk16f[:], pk16f_d[:])
            pk16b = sp.tile([16, PK16B_N], bf16)
            nc.scalar.dma_start(pk16b[:], pk16b_d[:])
            pkb = sp.tile([128, PKB_N], bf16)
            nc.scalar.dma_start(pkb[:], pkb_d[:])

            def pf(key, n):
                c = PKF_COLS[key]
                return pkf[:, c:c + n]

            def pb(key, n):
                c = PKB_COLS[key]
                return pkb[:, c:c + n]

            # ---- exchange buffer + remote descgen (early; source read
            # is deferred to the trigger) ----
            xrecv = sp.tile([128, 8 * 4], f32)
            for dlt in range(1, 8):
                rd = [None] * 8
                rd[dlt] = (0, dlt)
                nc.gpsimd.remote_dma_broadcast(
                    out_ap=xrecv[:, 4 * dlt:4 * dlt + 4],
                    in_ap=xrecv[:, 0:4],
                    remote_sem=rsem, local_sem=lsem, rdests=rd)

            # ---- bd9 zero-init (dep-free -> runs at t~0) ----
            bd9 = []
            for m in range(NMM):
                bdm = sp.tile([128, MWIDE], bf16, tag=f"bd9_{m}")
                nc.vector.memset(bdm[:].bitcast(mybir.dt.uint16), 0)
                bd9.append(bdm)

            # ---- GN stats: Square-accum on scalar, add-reduce on DVE ----
            stat4 = sp.tile([128, 4], f32)  # cols: 2*b + (0=sum, 1=sumsq)
            for b in range(B):
                sl = xe_slab[:, b * NSLAB:(b + 1) * NSLAB]
                st_sc = sp.tile([128, NSLAB], f32, tag="sc", bufs=1)
                nc.scalar.activation(st_sc[:], sl, AF.Square,
                                     accum_out=stat4[:, 2 * b + 1:2 * b + 2])
                nc.vector.tensor_reduce(stat4[:, 2 * b:2 * b + 1], sl,
                                        axis=mybir.AxisListType.X,
                                        op=ALU.add)

            # group-sum via mask matmul: [2, 4] (both groups core-local)
            g4 = hps.tile([2, 4], f32, tag="hps")
            nc.tensor.matmul(g4[:], pf("gstat", 2), stat4[:],
                             start=True, stop=True)
            gsum = sp.tile([2, 4], f32)
            nc.vector.tensor_copy(gsum[:], g4[:])

            # mu(neg), rsqrt(var+eps) per (group, b) -> mr4 [2,4]
            mr4 = sp.tile([2, 4], f32)  # cols: -mu0, -mu1, rs0, rs1
            nc.vector.tensor_scalar_mul(mr4[:, 0:2], gsum[:, 0:4:2],
                                        -1.0 / NGRP_ELEMS)
            ex2 = sp.tile([2, 2], f32)
            nc.vector.tensor_scalar_mul(ex2[:], gsum[:, 1:4:2],
                                        1.0 / NGRP_ELEMS)
            musq = sp.tile([2, 2], f32)
            nc.vector.tensor_mul(musq[:], mr4[:, 0:2], mr4[:, 0:2])
            vare = sp.tile([2, 2], f32)
            nc.vector.scalar_tensor_tensor(
                vare[:], ex2[:], 1.0, musq[:],
                op0=ALU.mult, op1=ALU.subtract)
            nc.vector.tensor_scalar_add(vare[:], vare[:], float(EPS))
            # Newton rsqrt (quake seed via float arithmetic, no tables)
            c1 = sp.tile([2, 2], f32)
            nc.vector.tensor_copy(c1[:], vare[:].bitcast(u32))
            sb = sp.tile([2, 2], f32)
            nc.vector.tensor_scalar(sb[:], c1[:], -0.5, 1597463007.0,
                                    op0=ALU.mult, op1=ALU.add)
            nc.vector.tensor_copy(mr4[:, 2:4].bitcast(u32), sb[:])
            hh = sp.tile([2, 2], f32)
            nc.vector.tensor_scalar_mul(hh[:], vare[:], 0.5)
            ycur = mr4[:, 2:4]
            for it in range(3):
                t2 = sp.tile([2, 2], f32, tag="nt", bufs=8)
                nc.vector.tensor_mul(t2[:], ycur, ycur)
                t3 = sp.tile([2, 2], f32, tag="nt", bufs=8)
                nc.vector.tensor_mul(t3[:], t2[:], hh[:])
                t4 = sp.tile([2, 2], f32, tag="nt", bufs=8)
                nc.vector.tensor_scalar(t4[:], t3[:], -1.0, 1.5,
                                        op0=ALU.mult, op1=ALU.add)
                ynew = sp.tile([2, 2], f32, tag="nt", bufs=8)
                nc.vector.tensor_mul(ynew[:], ycur, t4[:])
                nc.vector.tensor_copy(mr4[:, 2:4], ynew[:])
                ycur = mr4[:, 2:4]

            # expand groups -> (c,og) partitions: chmr [128,4]
            ch4 = hps.tile([128, 4], f32, tag="hps")
            nc.tensor.matmul(ch4[:], pk16f[0:2, 0:128], mr4[:],
                             start=True, stop=True)
            chmr = sp.tile([128, 4], f32)
            nc.vector.tensor_copy(chmr[:], ch4[:])
            # scale_c = rs*gamma ; bias_c = beta + (-mu)*scale
            scale = sp.tile([128, 2], f32)
            nc.vector.tensor_scalar_mul(scale[:], chmr[:, 2:4],
                                        pf("gnw", 2)[:, 0:1])
            nmus = sp.tile([128, 2], f32)
            nc.vector.tensor_mul(nmus[:], chmr[:, 0:2], scale[:])
            bias = sp.tile([128, 2], f32)
            nc.vector.tensor_scalar_add(bias[:], nmus[:],
                                        pf("gnw", 2)[:, 1:2])
            # negc = -bias/scale ; biasN = NWIN*bias
            rcs = sp.tile([128, 2], f32)
            nc.vector.reciprocal(rcs[:], scale[:])
            negc = sp.tile([128, 2], f32)
            nc.vector.scalar_tensor_tensor(
                negc[:], bias[:], -1.0, rcs[:], op0=ALU.mult, op1=ALU.mult)
            biasN = sp.tile([128, 2], f32)
            nc.vector.tensor_scalar_mul(biasN[:], bias[:], float(NWIN))

            # ---- window sums into S4 [128, 8] (cols b*4+s) ----
            # slots s=0,1,2 per b on DVE (max+accum), s=3 per b on scalar
            S4 = sp.tile([128, B * NSLOT], f32)
            acc4 = sp.tile([128, B * NSLOT], f32)
            for b in range(B):
                for s in range(NSLOT):
                    col = b * NSLOT + s
                    xs = xg[:, col * NWIN:(col + 1) * NWIN]
                    if s == 3:
                        rl_sc = sp.tile([128, NWIN], bf16, tag="sc2",
                                        bufs=1)
                        nc.scalar.activation(
                            rl_sc[:], xs, AF.Relu,
                            bias=bias[:, b:b + 1], scale=scale[:, b:b + 1],
                            accum_out=S4[:, col:col + 1])
                    else:
                        scr = sp.tile([128, NWIN], bf16, tag="scv",
                                      bufs=2)
                        nc.vector.tensor_scalar(
                            scr[:], xs, negc[:, b:b + 1], 0.0,
                            op0=ALU.max, op1=ALU.add,
                            accum_out=acc4[:, col:col + 1])
                # correct the DVE slots: S = scale*acc + NWIN*bias
                nc.vector.tensor_scalar(
                    S4[:, b * 4:b * 4 + 3], acc4[:, b * 4:b * 4 + 3],
                    scale[:, b:b + 1], biasN[:, b:b + 1],
                    op0=ALU.mult, op1=ALU.add)
            S4b = sp.tile([128, B * NSLOT], bf16)
            nc.vector.tensor_copy(S4b[:], S4[:])

            # ---- x_feat partials into xrecv slot 0: [128, 4] ----
            for oc in range(2):
                xfp = hps.tile([128, 2], f32, tag="hps")
                for s in range(NSLOT):
                    nc.tensor.matmul(
                        xfp[:],
                        pb("w2dt", 1024)[:, s * 256 + oc * 128:
                                         s * 256 + oc * 128 + 128],
                        S4b[:, s:s + NSLOT + 1:NSLOT],
                        start=(s == 0), stop=(s == NSLOT - 1))
                nc.vector.tensor_copy(xrecv[:, oc * 2:oc * 2 + 2], xfp[:])

            # fire the exchange (waits: descgen done + slot0 written +
            # prelude-AG, attached post-schedule)
            trig = nc.gpsimd.trigger_dma(count=None)
            nc._bir_kernel_barrier_sem_replica_groups.extend(
                [set(range(N_CORES))])

            # ---- HAM warm-up: keep PE busy during the exchange wait ----
            wps = hps.tile([8, 512], f32, tag="hps")
            for w in range(NWARM):
                nc.tensor.matmul(wps[:], S4b[:],
                                 pts[0][:, 0:512], start=True, stop=True)

            # ---- tree-sum the 8 slots (first add waits rsem>=14) ----
            s1 = sp.tile([128, 16], f32)
            sum1 = nc.vector.tensor_add(s1[:], xrecv[:, 0:16],
                                        xrecv[:, 16:32])
            s2 = sp.tile([128, 8], f32)
            nc.vector.tensor_add(s2[:], s1[:, 0:8], s1[:, 8:16])
            xfr = sp.tile([128, 4], f32)
            nc.vector.tensor_add(xfr[:], s2[:, 0:4], s2[:, 4:8])
            xfb = sp.tile([128, 4], f32)
            nc.vector.tensor_add(xfb[:], xfr[:], pf("gapbT", 4))

            # ---- xcT for feature half: [128, 12] cols pc*6 + (3b+k) ----
            xcT = sp.tile([128, 12], bf16)
            for pc in range(2):
                for b in range(B):
                    nc.vector.tensor_scalar_mul(
                        xcT[:, pc * 6 + 3 * b: pc * 6 + 3 * b + 3],
                        pb("ones6", 6)[:, 0:3],
                        xfb[:, pc * 2 + b: pc * 2 + b + 1])

            # ---- MLP1: p6T = relu(Wx @ x_feat + (We@emb + b_cf)).T ----
            p1w = hps.tile([128, 24], f32, tag="hps")
            for oc in range(4):
                for pc in range(2):
                    nc.tensor.matmul(
                        p1w[:, oc * 6:oc * 6 + 6],
                        pb("w_cfT", 1024)[:, pc * 512 + oc * 128:
                                          pc * 512 + oc * 128 + 128],
                        xcT[:, pc * 6:pc * 6 + 6],
                        start=(pc == 0), stop=False)
                nc.tensor.matmul(
                    p1w[:, oc * 6:oc * 6 + 6],
                    pk16b[0:6, PK16B_COLS["bcf6"] + oc * 128:
                          PK16B_COLS["bcf6"] + (oc + 1) * 128],
                    pk16b[0:6, PK16B_COLS["id6"]:PK16B_COLS["id6"] + 6],
                    start=False, stop=True)
            p6T = sp.tile([128, 4 * 6], bf16)
            nc.scalar.activation(p6T[:], p1w[:], AF.Relu)

            # ---- MLP2: c6T [128, 2*6]  (b_c folded into ba1 on host) ----
            c1w = hps.tile([128, 12], f32, tag="hps")
            for oc in range(2):
                for pc in range(4):
                    nc.tensor.matmul(
                        c1w[:, oc * 6:oc * 6 + 6],
                        pb("w_cT", 1024)[:, pc * 256 + oc * 128:
                                         pc * 256 + oc * 128 + 128],
                        p6T[:, pc * 6:pc * 6 + 6],
                        start=(pc == 0), stop=(pc == 3))
            c6T = sp.tile([128, 2 * 6], bf16)
            nc.scalar.activation(c6T[:], c1w[:], AF.Identity)

            # ---- MLP3: hT [16, 6] ----
            h1 = hps.tile([16, 6], f32, tag="hps")
            for pc in range(2):
                nc.tensor.matmul(h1[:],
                                 pb("w_a1T", 32)[:, pc * 16:pc * 16 + 16],
                                 c6T[:, pc * 6:pc * 6 + 6],
                                 start=(pc == 0), stop=(pc == 1))
            hT = sp.tile([16, 6], bf16)
            nc.scalar.activation(hT[:], h1[:], AF.Relu,
                                 bias=pk16f[0:16, 128:129])

            # ---- MLP4 widened to 128 rows: gT [128, 6] = sigmoid(..) ----
            g1 = hps.tile([128, 6], f32, tag="hps")
            nc.tensor.matmul(
                g1[:],
                pk16b[0:16, PK16B_COLS["w_a2T4"]:PK16B_COLS["w_a2T4"] + 128],
                hT[:], start=True, stop=True)
            gT = sp.tile([128, 6], f32)
            nc.scalar.activation(gT[:], g1[:], AF.Sigmoid,
                                 bias=pf("ba2_4", 1)[:, 0:1])

            # ---- effw + batch select, on all 128 partitions ----
            selR = sp.tile([128, 3], f32)
            nc.vector.scalar_tensor_tensor(
                selR[:], gT[:, 3:6], pf("msel", 2)[:, 1:2],
                pf("wseg", 6)[:, 3:6], op0=ALU.mult, op1=ALU.mult)
            selL = sp.tile([128, 3], f32)
            nc.vector.scalar_tensor_tensor(
                selL[:], gT[:, 0:3], pf("msel", 2)[:, 0:1],
                pf("wseg", 6)[:, 0:3], op0=ALU.mult, op1=ALU.mult)
            effB = sp.tile([128, 3], f32)
            nc.vector.tensor_add(effB[:], selL[:], selR[:])

            # bd9[0] block: bd[32g+c, 3g+k] = effB[32g+c, k]
            for g in range(G):
                nc.vector.tensor_scalar_mul(
                    bd9[0][:, 3 * g:3 * g + 3], effB[:],
                    pf("gsel", 4)[:, g:g + 1])
            # column-shifted replicas for the other 8 matmul slots
            for m in range(1, NMM):
                if m % 2 == 0:
                    nc.vector.tensor_copy(bd9[m][:, 12 * m:12 * m + 12],
                                          bd9[0][:, 0:12])
                else:
                    nc.scalar.activation(bd9[m][:, 12 * m:12 * m + 12],
                                         bd9[0][:, 0:12], AF.Copy)

          # ---- main stream: grouped tiles share each bd9[m] ----
          bseg_ap = pkf[0:MWIDE, PKF_COLS["bseg"]:PKF_COLS["bseg"] + 1]
          for (t0, t1) in GROUPS:
            pos = []
            for t in range(t0, t1):
                pot = sps.tile([MWIDE, 512], f32, tag="po", name=f"po{t}")
                pos.append(pot)
            for m in range(NMM):
                for i, t in enumerate(range(t0, t1)):
                    nc.tensor.matmul(pos[i][:], bd9[m][:],
                                     pts[t][:, m * 512:(m + 1) * 512],
                                     start=(m == 0), stop=(m == NMM - 1))
            for i, t in enumerate(range(t0, t1)):
                so = sp.tile([MWIDE, 512], bf16, tag="so", bufs=4)
                if t < NITER - 1:
                    if t % 2 == 0:
                        nc.scalar.activation(so[:], pos[i][:], AF.Identity,
                                             bias=bseg_ap)
                    else:
                        nc.vector.tensor_scalar_add(so[:], pos[i][:],
                                                    bseg_ap)
                    nc.gpsimd.dma_start(out_d[t], so[:])
                else:
                    # last tile: split across engines/queues for the tail
                    nc.scalar.activation(so[:, 0:256], pos[i][:, 0:256],
                                         AF.Identity, bias=bseg_ap)
                    nc.gpsimd.dma_start(out_d[t, :, 0:256], so[:, 0:256])
                    nc.vector.tensor_scalar_add(so[:, 256:512],
                                                pos[i][:, 256:512],
                                                bseg_ap)
                    nc.sync.dma_start(out_d[t, :, 256:512], so[:, 256:512])

    # cross-core waits attached after tile scheduling (the scheduler's
    # single-core sim cannot model remote increments)
    trig.wait_op(nc._bir_kernel_barrier_sem, 1, "sem-ge", check=False)
    sum1.wait_op(rsem, 14, "sem-ge", check=False)
    nc.compile()
    return nc


def _prep_shared(inp):
    """Host-side weight transposes/packing shared by all cores."""
    gn_g = np.asarray(inp["gn_g"], np.float32)
    gn_b = np.asarray(inp["gn_b"], np.float32)
    gap_b = np.asarray(inp["gap_b"], np.float32)
    w_cf = np.asarray(inp["w_cf"], np.float32)
    b_cf = np.asarray(inp["b_cf"], np.float32)
    w_c = np.asarray(inp["w_c"], np.float32)
    b_c = np.asarray(inp["b_c"], np.float32)
    w_a1 = np.asarray(inp["w_a1"], np.float32)
    b_a1 = np.asarray(inp["b_a1"], np.float32)
    w_a2 = np.asarray(inp["w_a2"], np.float32)
    b_a2 = np.asarray(inp["b_a2"], np.float32)
    emb = np.asarray(inp["emb"], np.float32)
    w_seg = np.asarray(inp["w_seg"], np.float32)
    b_seg = np.asarray(inp["b_seg"], np.float32)

    p = np.arange(128)
    j = np.arange(6)

    # ---- pkf (f32 [128, PKF_N]) except per-core msel/gnw ----
    pkf = np.zeros((128, PKF_N), np.float32)
    pkf[:, 0:2] = (p[:, None] // 64 == np.arange(2)[None, :])
    pkf[:, PKF_COLS["gapbT"]:PKF_COLS["gapbT"] + 4] = np.repeat(
        gap_b.reshape(2, 128).T, 2, axis=1)
    pkf[:, PKF_COLS["gsel"]:PKF_COLS["gsel"] + 4] = (
        p[:, None] // 32 == np.arange(4)[None, :])
    pkf[:, PKF_COLS["wseg"]:PKF_COLS["wseg"] + 6] = np.tile(
        w_seg[j % 3, :].T, (4, 1))
    pkf[:, PKF_COLS["ba2_4"]] = np.tile(b_a2, 4)
    pkf[0:MWIDE, PKF_COLS["bseg"]] = np.tile(b_seg, 36)

    # ---- pkb (bf16 [128, PKB_N]); w2dt filled per-core ----
    pkb = np.zeros((128, PKB_N), np.float32)
    wx = w_cf[:, 0:256].T                            # [256, 512]
    pkb[:, PKB_COLS["w_cfT"]:PKB_COLS["w_cfT"] + 1024] = np.concatenate(
        [wx[128 * pc:128 * (pc + 1), :] for pc in range(2)], axis=1)
    pkb[:, PKB_COLS["w_cT"]:PKB_COLS["w_cT"] + 1024] = np.concatenate(
        [w_c.T[128 * pc:128 * (pc + 1), :] for pc in range(4)], axis=1)
    pkb[:, PKB_COLS["w_a1T"]:PKB_COLS["w_a1T"] + 32] = np.concatenate(
        [w_a1.T[128 * pc:128 * (pc + 1), :] for pc in range(2)], axis=1)
    pkb[:, PKB_COLS["ones6"]:PKB_COLS["ones6"] + 6] = 1.0

    # ---- pk16f (f32 [16, PK16F_N]) ----
    pk16f = np.zeros((16, PK16F_N), np.float32)
    pk16f[0:2, 0:128] = pkf[:, 0:2].T               # gexp
    pk16f[:, 128] = b_a1 + w_a1 @ b_c               # ba1 (b_c folded)

    # ---- pk16b (bf16 [16, PK16B_N]) ----
    pk16b = np.zeros((16, PK16B_N), np.float32)
    pk16b[0:6, PK16B_COLS["bcf6"]:PK16B_COLS["bcf6"] + 512] = (
        b_cf[None, :] + emb[j % 3] @ w_cf[:, 256:512].T)
    pk16b[0:6, PK16B_COLS["id6"]:PK16B_COLS["id6"] + 6] = np.eye(6)
    pk16b[0:16, PK16B_COLS["w_a2T4"]:PK16B_COLS["w_a2T4"] + 128] = np.tile(
        w_a2.T, (1, 4))

    return pkf, pkb, pk16f.astype(np.float32), pk16b.astype(NP_BF16)


def kernel(**inputs):
    global LAST_EXEC_NS
    x_e = np.asarray(inputs["x_e"], np.float32)
    pred = np.asarray(inputs["pred"], np.float32)
    gap_w = np.asarray(inputs["gap_w"], np.float32)
    gn_g = np.asarray(inputs["gn_g"], np.float32)
    gn_b = np.asarray(inputs["gn_b"], np.float32)

    pkf0, pkb0, pk16f, pk16b = _prep_shared(inputs)

    # (og, s) -> conv offset table, identical on every core
    offs = [(4 * og + s) % 27 for og in range(8) for s in range(NSLOT)]
    cnt = np.bincount(np.array(offs), minlength=27).astype(np.float32)
    w2 = gap_w.reshape(256, 128, 27)

    # all 27 strided windows of x_e, gathered once: [27, B, 128, NWIN]
    wins = np.empty((27, B, 128, NWIN), np.float32)
    for o in range(27):
        kd, kw, kh = o // 9, (o // 3) % 3, o % 3
        win = x_e[:, :, kd:kd + 21:2, kw:kw + 21:2, kh:kh + 21:2]
        wins[o] = win.reshape(B, 128, NWIN)

    in_maps = []
    for r in range(N_CORES):
        b, dq = divmod(r, 4)
        ch = slice(16 * r, 16 * r + 16)

        ps = pred[b, :, dq * 24:(dq + 1) * 24]          # [32,24,96,96]
        ps = ps.reshape(32, G, NITER, COLS).transpose(2, 1, 0, 3)
        m = dict(
            pred_s=np.ascontiguousarray(
                ps.reshape(NITER, 128, COLS).astype(NP_BF16)),
            pk16f=pk16f, pk16b=pk16b)

        # stats slab: partitions (c:16, dchunk:8), cols b*1728 + pos
        sl = x_e[:, ch].reshape(B, 16, 8, NSLAB)
        m["xe_slab"] = np.ascontiguousarray(
            sl.transpose(1, 2, 0, 3).reshape(128, -1).astype(NP_BF16))

        # window gather: partitions (c:16, og:8), cols (b, s, pos)
        xgr = np.empty((16, 8, B, NSLOT, NWIN), np.float32)
        w2dt = np.empty((16, 8, NSLOT, 256), np.float32)
        for og in range(8):
            for sidx in range(NSLOT):
                o = offs[og * NSLOT + sidx]
                xgr[:, og, :, sidx, :] = wins[o][:, ch].transpose(1, 0, 2)
                w2dt[:, og, sidx, :] = (
                    w2[:, ch, o].T / np.float32(1331.0 * cnt[o]))
        m["xg"] = np.ascontiguousarray(
            xgr.reshape(128, -1).astype(NP_BF16))

        pkb = pkb0.copy()
        pkb[:, PKB_COLS["w2dt"]:PKB_COLS["w2dt"] + 1024] = (
            w2dt.reshape(128, -1))
        m["pkb"] = np.ascontiguousarray(pkb.astype(NP_BF16))

        pkf = pkf0.copy()
        # per-(c,og) gamma/beta
        pkf[:, PKF_COLS["gnw"]] = np.repeat(gn_g[ch], 8)
        pkf[:, PKF_COLS["gnw"] + 1] = np.repeat(gn_b[ch], 8)
        pkf[:, PKF_COLS["msel"] + b] = 1.0
        m["pkf"] = np.ascontiguousarray(pkf)
        in_maps.append(m)

    if "nc" not in _CACHE:
        _CACHE["nc"] = _build_program()
    nc = _CACHE["nc"]

    res = run_bass_kernel_spmd(nc, in_maps, list(range(N_CORES)),
                               trace=TRACE)
    LAST_EXEC_NS = res.exec_time_ns

    out = np.empty((B, K, 96, 96, 96), np.float32)
    for r in range(N_CORES):
        b, dq = divmod(r, 4)
        o = res.results[r]["out_s"].astype(np.float32)   # [12, 108, 512]
        o = o.reshape(NITER, NMM, G, K, 512)             # (t, m, g, k, j)
        o = o.transpose(3, 2, 0, 1, 4).reshape(K, NPOS)  # k, (g,t,m,j)
        out[b, :, dq * 24:(dq + 1) * 24] = o.reshape(K, 24, 96, 96)
    return out


# revision 11
# speedup vs baseline: 1.2050x; 1.2050x over previous
"""Trainium2 Bass kernel for nn_CLIP_3v3d_brats (dense_cnn head + gated 1x1 conv).

Sharding: 8 cores = batch(2) x 4 D-slabs of `pred`. The dominant einsum
logits[b,k,:] = sum_c effw[b,k,c]*pred[b,c,:] runs as a block-diagonal
bf16 matmul (4 position groups x 32 channels -> K=128 contraction).

v3 architecture (vs the 128us AllReduce baseline):
- The 2KB x_feat cross-core reduction uses remote_dma_broadcast
  (SBUF->SBUF, XOR-relative dests, sem-signaled) instead of a cold
  ncfw AllReduce: each core broadcasts its [128,4] partial into peer
  slot buffers and tree-sums the 8 slots on arrival (wait rsem>=14).
  Safety: a 1-byte prelude AllGather (bir_kernel_barrier) runs from
  t~0 concurrently with the head compute; the send trigger waits on
  it, so no send can race a slow core's semaphore-clear.
- Head chain has exactly ONE ACT table load: Sqrt is replaced by a
  float-trick Newton rsqrt on DVE, so Square/Relu/Identity/Sigmoid all
  live in one resident set.
- GN stats split scalar/DVE (Square-accum on scalar, add-reduce on
  DVE); window sums use the identity relu(s*x+b) = s*(max(x,-b/s))+b:
  one tensor_scalar(max,accum) pass per slot on DVE for 6 of 8 slots,
  scalar ACT(Relu,scale,bias,accum) for the other 2.
- Stream: 9 column-shifted block-diag stationaries, but tiles run in
  GROUPS (6,4,1,1) with the m-loop outermost inside a group so each
  LDWEIGHTS is reused across the group's PSUM tiles (18 LDW total,
  the rest issue at the pure ~107ns/512col bf16 rate). Dummy matmuls
  pinned after the x_feat partials keep/flip the HAM clock gate ON
  during the exchange wait so the stream runs at 2.4GHz.
- Output copies + DMA are bf16 (halves PSUM-copy write and out wire).
- Small weights packed into 4 tensors on the scalar DMA queue; pred
  tiles stream on sync immediately after xe_slab+xg halves.
"""
import sys
import types

sys.path.insert(0, "/opt/trn_rl_repo")

import numpy as np

try:
    import antenv.axon_hooks  # noqa: F401
except ImportError:
    try:
        import trn_agent_boot.trn_boot as _tb

        _hooks = types.ModuleType("antenv.axon_hooks")
        _the_hook = _tb._ntff_profile_via_ctypes("/opt/axon/libaxon_pjrt.so")
        _hooks.get_axon_ntff_profile_hook = lambda: _the_hook
        _hooks.set_axon_ntff_profile_hook = lambda h: None
        sys.modules["antenv.axon_hooks"] = _hooks
    except Exception:
        pass

from concourse import bacc, tile, mybir
from concourse.bass_utils import run_bass_kernel_spmd

f32 = mybir.dt.float32
bf16 = mybir.dt.bfloat16
u32 = mybir.dt.uint32
AF = mybir.ActivationFunctionType
ALU = mybir.AluOpType

NP_BF16 = mybir.dt.np(bf16)

N_CORES = 8
B = 2
K = 3
EPS = 1e-5
G = 4                      # position groups interleaved on partitions
NPOS = 221184              # positions per core slab: 24*96*96
NG = NPOS // G             # 55296
COLS = 4608                # stream iteration columns (9 matmuls of 512)
NITER = NG // COLS         # 12
NMM = COLS // 512          # 9
MWIDE = 12 * NMM           # 108 output partitions
NSLOT = 4                  # window-sum slots per core
NWIN = 1331                # 11^3 window positions per offset
NSLAB = 1728               # 3*24*24 stats-slab positions per batch
NGRP_ELEMS = 8 * 13824     # elements per (batch, group) for GN stats
NWARM = 12                 # HAM warm-up matmuls during the exchange wait
GROUPS = [(0, 6), (6, 12)]  # stream tile groups (all tiles resident)

# packed-constant column offsets (f32 [128, *])
PKF_COLS = dict(gstat=0, gnw=2, gapbT=4, msel=8, gsel=10, wseg=14,
                ba2_4=20, bseg=21)
PKF_N = 22
# packed bf16 [128, *]
PKB_COLS = dict(w2dt=0, w_cfT=1024, w_cT=2048, w_a1T=3072, ones6=3104)
PKB_N = 3110
# packed f32 [16, *]: gexp rows0-1 cols0-127, ba1 col 128
PK16F_N = 129
# packed bf16 [16, *]: bcf6 512, id6 6, w_a2T4 128
PK16B_COLS = dict(bcf6=0, id6=512, w_a2T4=518)
PK16B_N = 646

TRACE = False
LAST_EXEC_NS = None
_CACHE = {}


def _build_program():
    nc = bacc.Bacc("TRN2", target_bir_lowering=False, debug=False,
                   num_devices=N_CORES)

    def din(name, shape, dt=f32):
        return nc.dram_tensor(name, shape, dt, kind="ExternalInput").ap()

    pred_s = din("pred_s", [NITER, 128, COLS], bf16)
    xe_slab_d = din("xe_slab", [128, B * NSLAB], bf16)
    xg_d = din("xg", [128, B * NSLOT * NWIN], bf16)
    pkf_d = din("pkf", [128, PKF_N])
    pkb_d = din("pkb", [128, PKB_N], bf16)
    pk16f_d = din("pk16f", [16, PK16F_N])
    pk16b_d = din("pk16b", [16, PK16B_N], bf16)

    out_d = nc.dram_tensor("out_s", [NITER, MWIDE, 512], bf16,
                           kind="ExternalOutput").ap()

    with tile.TileContext(nc) as tc:
        with tc.tile_pool(name="small", bufs=1) as sp, \
             tc.tile_pool(name="hps", bufs=2, space="PSUM") as hps, \
             tc.tile_pool(name="sps", bufs=6, space="PSUM") as sps, \
             tc.tile_pool(name="dram", bufs=1, space="DRAM") as dram:
          if True:
            # ---- loads: sync queue = xe_slab, xg halves, pred tiles ----
            xe_slab = sp.tile([128, B * NSLAB], bf16)
            nc.sync.dma_start(xe_slab[:], xe_slab_d[:])
            HW = NSLOT * NWIN
            xg = sp.tile([128, B * HW], bf16)
            nc.sync.dma_start(xg[:, 0:HW], xg_d[:, 0:HW])
            nc.sync.dma_start(xg[:, HW:2 * HW], xg_d[:, HW:2 * HW])
            pts = []
            for t in range(NITER):
                pt = sp.tile([128, COLS], bf16, tag="pt", bufs=NITER)
                nc.sync.dma_start(pt[:], pred_s[t])
                pts.append(pt)
            # scalar queue: packed weights
            pkf = sp.tile([128, PKF_N], f32)
            nc.scalar.dma_start(pkf[:], pkf_d[:])
            pk16f = sp.tile([16, PK16F_N], f32)
            nc.scalar.dma_start(pk16f[:], pk16f_d[:])
            pk16b = sp.tile([16, PK16B_N], bf16)
            nc.scalar.dma_start(pk16b[:], pk16b_d[:])
            pkb = sp.tile([128, PKB_N], bf16)
            nc.scalar.dma_start(pkb[:], pkb_d[:])

            def pf(key, n):
                c = PKF_COLS[key]
                return pkf[:, c:c + n]

            def pb(key, n):
                c = PKB_COLS[key]
                return pkb[:, c:c + n]

            # ---- bd9 zero-init (dep-free -> runs at t~0) ----
            bd9 = []
            for m in range(NMM):
                bdm = sp.tile([128, MWIDE], bf16, tag=f"bd9_{m}")
                nc.vector.memset(bdm[:].bitcast(mybir.dt.uint16), 0)
                bd9.append(bdm)

            # ---- GN stats: Square-accum on scalar, add-reduce on DVE ----
            stat4 = sp.tile([128, 4], f32)  # cols: 2*b + (0=sum, 1=sumsq)
            for b in range(B):
                sl = xe_slab[:, b * NSLAB:(b + 1) * NSLAB]
                st_sc = sp.tile([128, NSLAB], f32, tag="sc", bufs=1)
                nc.scalar.activation(st_sc[:], sl, AF.Square,
                                     accum_out=stat4[:, 2 * b + 1:2 * b + 2])
                nc.vector.tensor_reduce(stat4[:, 2 * b:2 * b + 1], sl,
                                        axis=mybir.AxisListType.X,
                                        op=ALU.add)

            # group-sum via mask matmul: [2, 4] (both groups core-local)
            g4 = hps.tile([2, 4], f32, tag="hps")
            nc.tensor.matmul(g4[:], pf("gstat", 2), stat4[:],
                             start=True, stop=True)
            gsum = sp.tile([2, 4], f32)
            nc.vector.tensor_copy(gsum[:], g4[:])

            # mu(neg), rsqrt(var+eps) per (group, b) -> mr4 [2,4]
            mr4 = sp.tile([2, 4], f32)  # cols: -mu0, -mu1, rs0, rs1
            nc.vector.tensor_scalar_mul(mr4[:, 0:2], gsum[:, 0:4:2],
                                        -1.0 / NGRP_ELEMS)
            ex2 = sp.tile([2, 2], f32)
            nc.vector.tensor_scalar_mul(ex2[:], gsum[:, 1:4:2],
                                        1.0 / NGRP_ELEMS)
            musq = sp.tile([2, 2], f32)
            nc.vector.tensor_mul(musq[:], mr4[:, 0:2], mr4[:, 0:2])
            vare = sp.tile([2, 2], f32)
            nc.vector.scalar_tensor_tensor(
                vare[:], ex2[:], 1.0, musq[:],
                op0=ALU.mult, op1=ALU.subtract)
            nc.vector.tensor_scalar_add(vare[:], vare[:], float(EPS))
            # Newton rsqrt (quake seed via float arithmetic, no tables)
            c1 = sp.tile([2, 2], f32)
            nc.vector.tensor_copy(c1[:], vare[:].bitcast(u32))
            sb = sp.tile([2, 2], f32)
            nc.vector.tensor_scalar(sb[:], c1[:], -0.5, 1597463007.0,
                                    op0=ALU.mult, op1=ALU.add)
            nc.vector.tensor_copy(mr4[:, 2:4].bitcast(u32), sb[:])
            hh = sp.tile([2, 2], f32)
            nc.vector.tensor_scalar_mul(hh[:], vare[:], 0.5)
            ycur = mr4[:, 2:4]
            for it in range(3):
                t2 = sp.tile([2, 2], f32, tag="nt", bufs=8)
                nc.vector.tensor_mul(t2[:], ycur, ycur)
                t3 = sp.tile([2, 2], f32, tag="nt", bufs=8)
                nc.vector.tensor_mul(t3[:], t2[:], hh[:])
                t4 = sp.tile([2, 2], f32, tag="nt", bufs=8)
                nc.vector.tensor_scalar(t4[:], t3[:], -1.0, 1.5,
                                        op0=ALU.mult, op1=ALU.add)
                ynew = sp.tile([2, 2], f32, tag="nt", bufs=8)
                nc.vector.tensor_mul(ynew[:], ycur, t4[:])
                nc.vector.tensor_copy(mr4[:, 2:4], ynew[:])
                ycur = mr4[:, 2:4]

            # expand groups -> (c,og) partitions: chmr [128,4]
            ch4 = hps.tile([128, 4], f32, tag="hps")
            nc.tensor.matmul(ch4[:], pk16f[0:2, 0:128], mr4[:],
                             start=True, stop=True)
            chmr = sp.tile([128, 4], f32)
            nc.vector.tensor_copy(chmr[:], ch4[:])
            # scale_c = rs*gamma ; bias_c = beta + (-mu)*scale
            scale = sp.tile([128, 2], f32)
            nc.vector.tensor_scalar_mul(scale[:], chmr[:, 2:4],
                                        pf("gnw", 2)[:, 0:1])
            nmus = sp.tile([128, 2], f32)
            nc.vector.tensor_mul(nmus[:], chmr[:, 0:2], scale[:])
            bias = sp.tile([128, 2], f32)
            nc.vector.tensor_scalar_add(bias[:], nmus[:],
                                        pf("gnw", 2)[:, 1:2])
            # negc = -bias/scale ; biasN = NWIN*bias
            rcs = sp.tile([128, 2], f32)
            nc.vector.reciprocal(rcs[:], scale[:])
            negc = sp.tile([128, 2], f32)
            nc.vector.scalar_tensor_tensor(
                negc[:], bias[:], -1.0, rcs[:], op0=ALU.mult, op1=ALU.mult)
            biasN = sp.tile([128, 2], f32)
            nc.vector.tensor_scalar_mul(biasN[:], bias[:], float(NWIN))

            # ---- window sums into S4 [128, 8] (cols b*4+s) ----
            # slots s=0,1,2 per b on DVE (max+accum), s=3 per b on scalar
            S4 = sp.tile([128, B * NSLOT], f32)
            acc4 = sp.tile([128, B * NSLOT], f32)
            for b in range(B):
                for s in range(NSLOT):
                    col = b * NSLOT + s
                    xs = xg[:, col * NWIN:(col + 1) * NWIN]
                    if s == 3:
                        rl_sc = sp.tile([128, NWIN], bf16, tag="sc2",
                                        bufs=1)
                        nc.scalar.activation(
                            rl_sc[:], xs, AF.Relu,
                            bias=bias[:, b:b + 1], scale=scale[:, b:b + 1],
                            accum_out=S4[:, col:col + 1])
                    else:
                        scr = sp.tile([128, NWIN], bf16, tag="scv",
                                      bufs=2)
                        nc.vector.tensor_scalar(
                            scr[:], xs, negc[:, b:b + 1], 0.0,
                            op0=ALU.max, op1=ALU.add,
                            accum_out=acc4[:, col:col + 1])
                # correct the DVE slots: S = scale*acc + NWIN*bias
                nc.vector.tensor_scalar(
                    S4[:, b * 4:b * 4 + 3], acc4[:, b * 4:b * 4 + 3],
                    scale[:, b:b + 1], biasN[:, b:b + 1],
                    op0=ALU.mult, op1=ALU.add)
            S4b = sp.tile([128, B * NSLOT], bf16)
            nc.vector.tensor_copy(S4b[:], S4[:])

            # ---- x_feat partials: xfs [128, 4] ----
            xfs = sp.tile([128, 4], f32)
            for oc in range(2):
                xfp = hps.tile([128, 2], f32, tag="hps")
                for s in range(NSLOT):
                    nc.tensor.matmul(
                        xfp[:],
                        pb("w2dt", 1024)[:, s * 256 + oc * 128:
                                         s * 256 + oc * 128 + 128],
                        S4b[:, s:s + NSLOT + 1:NSLOT],
                        start=(s == 0), stop=(s == NSLOT - 1))
                nc.vector.tensor_copy(xfs[:, oc * 2:oc * 2 + 2], xfp[:])

            # ---- the one collective: 8-rank AllReduce of [128,4] ----
            ar_in = dram.tile([128, 4], f32)
            ar_out = dram.tile([128, 4], f32)
            nc.scalar.dma_start(ar_in[:], xfs[:])
            nc.gpsimd.collective_compute(
                "AllReduce", ALU.add,
                replica_groups=[list(range(N_CORES))],
                ins=[ar_in.opt()], outs=[ar_out.opt()])

            # ---- HAM warm-up: keep PE busy during the AR wait ----
            wps = hps.tile([8, 512], f32, tag="hps")
            for w in range(NWARM):
                nc.tensor.matmul(wps[:], S4b[:],
                                 pts[0][:, 0:512], start=True, stop=True)

            xfr = sp.tile([128, 4], f32)
            nc.scalar.dma_start(xfr[:], ar_out[:])
            xfb = sp.tile([128, 4], f32)
            nc.vector.tensor_add(xfb[:], xfr[:], pf("gapbT", 4))

            # ---- xcT for feature half: [128, 12] cols pc*6 + (3b+k) ----
            xcT = sp.tile([128, 12], bf16)
            for pc in range(2):
                for b in range(B):
                    nc.vector.tensor_scalar_mul(
                        xcT[:, pc * 6 + 3 * b: pc * 6 + 3 * b + 3],
                        pb("ones6", 6)[:, 0:3],
                        xfb[:, pc * 2 + b: pc * 2 + b + 1])

            # ---- MLP1: p6T = relu(Wx @ x_feat + (We@emb + b_cf)).T ----
            p1w = hps.tile([128, 24], f32, tag="hps")
            for oc in range(4):
                for pc in range(2):
                    nc.tensor.matmul(
                        p1w[:, oc * 6:oc * 6 + 6],
                        pb("w_cfT", 1024)[:, pc * 512 + oc * 128:
                                          pc * 512 + oc * 128 + 128],
                        xcT[:, pc * 6:pc * 6 + 6],
                        start=(pc == 0), stop=False)
                nc.tensor.matmul(
                    p1w[:, oc * 6:oc * 6 + 6],
                    pk16b[0:6, PK16B_COLS["bcf6"] + oc * 128:
                          PK16B_COLS["bcf6"] + (oc + 1) * 128],
                    pk16b[0:6, PK16B_COLS["id6"]:PK16B_COLS["id6"] + 6],
                    start=False, stop=True)
            p6T = sp.tile([128, 4 * 6], bf16)
            nc.scalar.activation(p6T[:], p1w[:], AF.Relu)

            # ---- MLP2: c6T [128, 2*6]  (b_c folded into ba1 on host) ----
            c1w = hps.tile([128, 12], f32, tag="hps")
            for oc in range(2):
                for pc in range(4):
                    nc.tensor.matmul(
                        c1w[:, oc * 6:oc * 6 + 6],
                        pb("w_cT", 1024)[:, pc * 256 + oc * 128:
                                         pc * 256 + oc * 128 + 128],
                        p6T[:, pc * 6:pc * 6 + 6],
                        start=(pc == 0), stop=(pc == 3))
            c6T = sp.tile([128, 2 * 6], bf16)
            nc.scalar.activation(c6T[:], c1w[:], AF.Identity)

            # ---- MLP3: hT [16, 6] ----
            h1 = hps.tile([16, 6], f32, tag="hps")
            for pc in range(2):
                nc.tensor.matmul(h1[:],
                                 pb("w_a1T", 32)[:, pc * 16:pc * 16 + 16],
                                 c6T[:, pc * 6:pc * 6 + 6],
                                 start=(pc == 0), stop=(pc == 1))
            hT = sp.tile([16, 6], bf16)
            nc.scalar.activation(hT[:], h1[:], AF.Relu,
                                 bias=pk16f[0:16, 128:129])

            # ---- MLP4 widened to 128 rows: gT [128, 6] = sigmoid(..) ----
            g1 = hps.tile([128, 6], f32, tag="hps")
            nc.tensor.matmul(
                g1[:],
                pk16b[0:16, PK16B_COLS["w_a2T4"]:PK16B_COLS["w_a2T4"] + 128],
                hT[:], start=True, stop=True)
            gT = sp.tile([128, 6], f32)
            nc.scalar.activation(gT[:], g1[:], AF.Sigmoid,
                                 bias=pf("ba2_4", 1)[:, 0:1])

            # ---- effw + batch select, on all 128 partitions ----
            selR = sp.tile([128, 3], f32)
            nc.vector.scalar_tensor_tensor(
                selR[:], gT[:, 3:6], pf("msel", 2)[:, 1:2],
                pf("wseg", 6)[:, 3:6], op0=ALU.mult, op1=ALU.mult)
            selL = sp.tile([128, 3], f32)
            nc.vector.scalar_tensor_tensor(
                selL[:], gT[:, 0:3], pf("msel", 2)[:, 0:1],
                pf("wseg", 6)[:, 0:3], op0=ALU.mult, op1=ALU.mult)
            effB = sp.tile([128, 3], f32)
            nc.vector.tensor_add(effB[:], selL[:], selR[:])

            # bd9[0] block: bd[32g+c, 3g+k] = effB[32g+c, k]
            for g in range(G):
                nc.vector.tensor_scalar_mul(
                    bd9[0][:, 3 * g:3 * g + 3], effB[:],
                    pf("gsel", 4)[:, g:g + 1])
            # column-shifted replicas for the other 8 matmul slots
            for m in range(1, NMM):
                if m % 2 == 0:
                    nc.vector.tensor_copy(bd9[m][:, 12 * m:12 * m + 12],
                                          bd9[0][:, 0:12])
                else:
                    nc.scalar.activation(bd9[m][:, 12 * m:12 * m + 12],
                                         bd9[0][:, 0:12], AF.Copy)

          # ---- main stream: grouped tiles share each bd9[m] ----
          bseg_ap = pkf[0:MWIDE, PKF_COLS["bseg"]:PKF_COLS["bseg"] + 1]
          for (t0, t1) in GROUPS:
            pos = []
            for t in range(t0, t1):
                pot = sps.tile([MWIDE, 512], f32, tag="po", name=f"po{t}")
                pos.append(pot)
            for m in range(NMM):
                for i, t in enumerate(range(t0, t1)):
                    nc.tensor.matmul(pos[i][:], bd9[m][:],
                                     pts[t][:, m * 512:(m + 1) * 512],
                                     start=(m == 0), stop=(m == NMM - 1))
            for i, t in enumerate(range(t0, t1)):
                so = sp.tile([MWIDE, 512], bf16, tag="so", bufs=4)
                if t < NITER - 1:
                    if t % 2 == 0:
                        nc.scalar.activation(so[:], pos[i][:], AF.Identity,
                                             bias=bseg_ap)
                    else:
                        nc.vector.tensor_scalar_add(so[:], pos[i][:],
                                                    bseg_ap)
                    nc.gpsimd.dma_start(out_d[t], so[:])
                else:
                    # last tile: split across engines/queues for the tail
                    nc.scalar.activation(so[:, 0:256], pos[i][:, 0:256],
                                         AF.Identity, bias=bseg_ap)
                    nc.gpsimd.dma_start(out_d[t, :, 0:256], so[:, 0:256])
                    nc.vector.tensor_scalar_add(so[:, 256:512],
                                                pos[i][:, 256:512],
                                                bseg_ap)
                    nc.sync.dma_start(out_d[t, :, 256:512], so[:, 256:512])

    nc.compile()
    return nc


def _prep_shared(inp):
    """Host-side weight transposes/packing shared by all cores."""
    gn_g = np.asarray(inp["gn_g"], np.float32)
    gn_b = np.asarray(inp["gn_b"], np.float32)
    gap_b = np.asarray(inp["gap_b"], np.float32)
    w_cf = np.asarray(inp["w_cf"], np.float32)
    b_cf = np.asarray(inp["b_cf"], np.float32)
    w_c = np.asarray(inp["w_c"], np.float32)
    b_c = np.asarray(inp["b_c"], np.float32)
    w_a1 = np.asarray(inp["w_a1"], np.float32)
    b_a1 = np.asarray(inp["b_a1"], np.float32)
    w_a2 = np.asarray(inp["w_a2"], np.float32)
    b_a2 = np.asarray(inp["b_a2"], np.float32)
    emb = np.asarray(inp["emb"], np.float32)
    w_seg = np.asarray(inp["w_seg"], np.float32)
    b_seg = np.asarray(inp["b_seg"], np.float32)

    p = np.arange(128)
    j = np.arange(6)

    # ---- pkf (f32 [128, PKF_N]) except per-core msel/gnw ----
    pkf = np.zeros((128, PKF_N), np.float32)
    pkf[:, 0:2] = (p[:, None] // 64 == np.arange(2)[None, :])
    pkf[:, PKF_COLS["gapbT"]:PKF_COLS["gapbT"] + 4] = np.repeat(
        gap_b.reshape(2, 128).T, 2, axis=1)
    pkf[:, PKF_COLS["gsel"]:PKF_COLS["gsel"] + 4] = (
        p[:, None] // 32 == np.arange(4)[None, :])
    pkf[:, PKF_COLS["wseg"]:PKF_COLS["wseg"] + 6] = np.tile(
        w_seg[j % 3, :].T, (4, 1))
    pkf[:, PKF_COLS["ba2_4"]] = np.tile(b_a2, 4)
    pkf[0:MWIDE, PKF_COLS["bseg"]] = np.tile(b_seg, 36)

    # ---- pkb (bf16 [128, PKB_N]); w2dt filled per-core ----
    pkb = np.zeros((128, PKB_N), np.float32)
    wx = w_cf[:, 0:256].T                            # [256, 512]
    pkb[:, PKB_COLS["w_cfT"]:PKB_COLS["w_cfT"] + 1024] = np.concatenate(
        [wx[128 * pc:128 * (pc + 1), :] for pc in range(2)], axis=1)
    pkb[:, PKB_COLS["w_cT"]:PKB_COLS["w_cT"] + 1024] = np.concatenate(
        [w_c.T[128 * pc:128 * (pc + 1), :] for pc in range(4)], axis=1)
    pkb[:, PKB_COLS["w_a1T"]:PKB_COLS["w_a1T"] + 32] = np.concatenate(
        [w_a1.T[128 * pc:128 * (pc + 1), :] for pc in range(2)], axis=1)
    pkb[:, PKB_COLS["ones6"]:PKB_COLS["ones6"] + 6] = 1.0

    # ---- pk16f (f32 [16, PK16F_N]) ----
    pk16f = np.zeros((16, PK16F_N), np.float32)
    pk16f[0:2, 0:128] = pkf[:, 0:2].T               # gexp
    pk16f[:, 128] = b_a1 + w_a1 @ b_c               # ba1 (b_c folded)

    # ---- pk16b (bf16 [16, PK16B_N]) ----
    pk16b = np.zeros((16, PK16B_N), np.float32)
    pk16b[0:6, PK16B_COLS["bcf6"]:PK16B_COLS["bcf6"] + 512] = (
        b_cf[None, :] + emb[j % 3] @ w_cf[:, 256:512].T)
    pk16b[0:6, PK16B_COLS["id6"]:PK16B_COLS["id6"] + 6] = np.eye(6)
    pk16b[0:16, PK16B_COLS["w_a2T4"]:PK16B_COLS["w_a2T4"] + 128] = np.tile(
        w_a2.T, (1, 4))

    return pkf, pkb, pk16f.astype(np.float32), pk16b.astype(NP_BF16)


def kernel(**inputs):
    global LAST_EXEC_NS
    x_e = np.asarray(inputs["x_e"], np.float32)
    pred = np.asarray(inputs["pred"], np.float32)
    gap_w = np.asarray(inputs["gap_w"], np.float32)
    gn_g = np.asarray(inputs["gn_g"], np.float32)
    gn_b = np.asarray(inputs["gn_b"], np.float32)

    pkf0, pkb0, pk16f, pk16b = _prep_shared(inputs)

    # (og, s) -> conv offset table, identical on every core
    offs = [(4 * og + s) % 27 for og in range(8) for s in range(NSLOT)]
    cnt = np.bincount(np.array(offs), minlength=27).astype(np.float32)
    w2 = gap_w.reshape(256, 128, 27)

    # all 27 strided windows of x_e, gathered once: [27, B, 128, NWIN]
    wins = np.empty((27, B, 128, NWIN), np.float32)
    for o in range(27):
        kd, kw, kh = o // 9, (o // 3) % 3, o % 3
        win = x_e[:, :, kd:kd + 21:2, kw:kw + 21:2, kh:kh + 21:2]
        wins[o] = win.reshape(B, 128, NWIN)

    in_maps = []
    for r in range(N_CORES):
        b, dq = divmod(r, 4)
        ch = slice(16 * r, 16 * r + 16)

        ps = pred[b, :, dq * 24:(dq + 1) * 24]          # [32,24,96,96]
        ps = ps.reshape(32, G, NITER, COLS).transpose(2, 1, 0, 3)
        m = dict(
            pred_s=np.ascontiguousarray(
                ps.reshape(NITER, 128, COLS).astype(NP_BF16)),
            pk16f=pk16f, pk16b=pk16b)

        # stats slab: partitions (c:16, dchunk:8), cols b*1728 + pos
        sl = x_e[:, ch].reshape(B, 16, 8, NSLAB)
        m["xe_slab"] = np.ascontiguousarray(
            sl.transpose(1, 2, 0, 3).reshape(128, -1).astype(NP_BF16))

        # window gather: partitions (c:16, og:8), cols (b, s, pos)
        xgr = np.empty((16, 8, B, NSLOT, NWIN), np.float32)
        w2dt = np.empty((16, 8, NSLOT, 256), np.float32)
        for og in range(8):
            for sidx in range(NSLOT):
                o = offs[og * NSLOT + sidx]
                xgr[:, og, :, sidx, :] = wins[o][:, ch].transpose(1, 0, 2)
                w2dt[:, og, sidx, :] = (
                    w2[:, ch, o].T / np.float32(1331.0 * cnt[o]))
        m["xg"] = np.ascontiguousarray(
            xgr.reshape(128, -1).astype(NP_BF16))

        pkb = pkb0.copy()
        pkb[:, PKB_COLS["w2dt"]:PKB_COLS["w2dt"] + 1024] = (
            w2dt.reshape(128, -1))
        m["pkb"] = np.ascontiguousarray(pkb.astype(NP_BF16))

        pkf = pkf0.copy()
        # per-(c,og) gamma/beta
        pkf[:, PKF_COLS["gnw"]] = np.repeat(gn_g[ch], 8)
        pkf[:, PKF_COLS["gnw"] + 1] = np.repeat(gn_b[ch], 8)
        pkf[:, PKF_COLS["msel"] + b] = 1.0
        m["pkf"] = np.ascontiguousarray(pkf)
        in_maps.append(m)

    if "nc" not in _CACHE:
        _CACHE["nc"] = _build_program()
    nc = _CACHE["nc"]

    res = run_bass_kernel_spmd(nc, in_maps, list(range(N_CORES)),
                               trace=TRACE)
    LAST_EXEC_NS = res.exec_time_ns

    out = np.empty((B, K, 96, 96, 96), np.float32)
    for r in range(N_CORES):
        b, dq = divmod(r, 4)
        o = res.results[r]["out_s"].astype(np.float32)   # [12, 108, 512]
        o = o.reshape(NITER, NMM, G, K, 512)             # (t, m, g, k, j)
        o = o.transpose(3, 2, 0, 1, 4).reshape(K, NPOS)  # k, (g,t,m,j)
        out[b, :, dq * 24:(dq + 1) * 24] = o.reshape(K, 24, 96, 96)
    return out
